# revision 7
# baseline (speedup 1.0000x reference)
"""Block-sparse linear y = x @ W^T on 8 Trainium2 NeuronCores.

Strategy: the 32x32 block structure (50% block density, random scatter) is not
exploitable on a 128x128 PE array (M=32 tiles run at 25% utilization and the
per-block LDWEIGHTS cost dominates), so we densify W^T on the host (cheap: 8MB
of scatter-adds) and run a dense fp32 GEMM, sharded 4-way over tokens x 2-way
over out_features (8 cores, no collectives needed).

Per core: y_shard[1024, 1024] = xT_shard[2048, 1024]^T @ wT_shard[2048, 1024].
x is transposed on the host so both operands stream into SBUF with K (=in
features) on partitions in natural, fully-contiguous DMA layouts.

Matmul dtype: float32r (single-pass fp32 matmul, 1 cycle/row at N>=512) vs
float32 (2-pass, 4 cycles/row). Switch via MM_DTYPE.
"""

import numpy as np

TOKENS, IN_F, OUT_F = 4096, 2048, 2048
BLOCK = 32
N_CORES = 8
TG, OG = 4, 2  # token groups x out-feature groups
T_SH = TOKENS // TG  # 1024 tokens per core
O_SH = OUT_F // OG  # 1024 out features per core
P = 128
NFREE = 512  # PSUM bank free dim (fp32)
KT = IN_F // P  # 16 k tiles
MT = T_SH // P  # 8 psum row tiles (uses all 8 PSUM banks)
NT = O_SH // NFREE  # 2 out column tiles

MM_DTYPE = "float32r"  # "float32r" (fast) or "float32" (exact 2-pass)
TRACE = False  # set by test.py to capture an NTFF profile

_nc_cache = {}
_last_result = None  # BassKernelResults of the most recent run (for test.py)


def _build_nc():
    import concourse.mybir as mybir
    import concourse.tile as tile
    from concourse import bacc

    key = MM_DTYPE
    if key in _nc_cache:
        return _nc_cache[key]

    dt_mm = getattr(mybir.dt, MM_DTYPE)
    f32 = mybir.dt.float32

    nc = bacc.Bacc(None, target_bir_lowering=False)
    xT = nc.dram_tensor("xT", [IN_F, T_SH], dt_mm, kind="ExternalInput")
    wT = nc.dram_tensor("wT", [IN_F, O_SH], dt_mm, kind="ExternalInput")
    y = nc.dram_tensor("y", [T_SH, O_SH], f32, kind="ExternalOutput")

    # Schedule: quarters (n, kh) over out-halves n and K-halves kh, ordered
    # (0,A)(1,A)(0,B)(1,B); splitting K spreads the 8MB x^T load across the
    # first two quarters (the kernel sits at the DMA roofline). Pass-A psums
    # are evicted to SBUF partials; pass B adds them back on the way out.
    # Each quarter runs as two 4-bank octants (banks 0-3 then 4-7) so psum
    # eviction of one bank set always overlaps matmuls on the other set.
    # Pass B runs k-inner per bank so finished tiles drain immediately.
    # Streams use separate DMA queues (x+out: sync, W: scalar) so one
    # stream's pool-slot wait never blocks the other's FIFO.
    KH = KT // 2  # 8 k-tiles per half
    HB = MT // 2  # 4 banks per octant
    XH = T_SH // 2  # x^T tiles split in halves of 512 tokens
    with tile.TileContext(nc) as tc:
        with (
            tc.tile_pool(name="xp", bufs=1) as xp,
            tc.tile_pool(name="wp", bufs=2) as wp,
            tc.tile_pool(name="pp", bufs=1) as pp,
            tc.tile_pool(name="op", bufs=4) as op,
            tc.tile_pool(name="ps", bufs=1, space="PSUM") as ps,
        ):
            # x^T half-tiles: xh[h][k] covers tokens [h*512, (h+1)*512)
            xh = [[None] * KT, [None] * KT]

            def load_xh(h, k):
                t = xp.tile([P, XH], dt_mm, tag=f"x{h}_{k}", name=f"x{h}_{k}")
                nc.sync.dma_start(
                    t[:], xT[k * P : (k + 1) * P, h * XH : (h + 1) * XH]
                )
                xh[h][k] = t

            def lhsT(m, k):
                return xh[m // HB][k][:, (m % HB) * P : (m % HB + 1) * P]

            partials = {}
            for qi, (n, kh) in enumerate([(0, 0), (1, 0), (0, 1), (1, 1)]):
                # Quarter's 8 W tiles: dedicated slots, live across both
                # octants; bufs=2 lets the next quarter's loads pipeline.
                wts = []
                for ki in range(KH):
                    k = kh * KH + ki
                    wt = wp.tile(
                        [P, NFREE], dt_mm, tag=f"wt{ki}", name=f"wt{ki}"
                    )
                    nc.scalar.dma_start(
                        wt[:], wT[k * P : (k + 1) * P, n * NFREE : (n + 1) * NFREE]
                    )
                    wts.append(wt)
                psums = [
                    ps.tile([P, NFREE], f32, tag=f"ps{m}", name=f"ps{m}")
                    for m in range(MT)
                ]
                if kh == 0:  # pass A: k-outer over all 8 banks, x staged JIT
                    for ki in range(KH):
                        for h in range(2):
                            if qi == 0 and xh[h][ki] is None:
                                load_xh(h, ki)  # A-half of x, just in time
                            if qi == 1 and xh[h][KH + ki] is None:
                                load_xh(h, KH + ki)  # prefetch B half
                        for m in range(MT):
                            nc.tensor.matmul(
                                psums[m][:],
                                lhsT(m, ki),
                                wts[ki][:],
                                start=(ki == 0),
                                stop=(ki == KH - 1),
                            )
                    for m in range(MT):  # evict partial sums to SBUF
                        pt = pp.tile(
                            [P, NFREE], f32, tag=f"pt{n}_{m}", name=f"pt{n}_{m}"
                        )
                        nc.vector.tensor_copy(pt[:], psums[m][:])
                        partials[(n, m)] = pt
                else:  # pass B: k-inner per bank so finished banks drain early
                    for m in range(MT):
                        for ki in range(KH):
                            nc.tensor.matmul(
                                psums[m][:],
                                lhsT(m, KH + ki),
                                wts[ki][:],
                                start=(ki == 0),
                                stop=(ki == KH - 1),
                            )
                        ot = op.tile([P, NFREE], f32, tag="ot")
                        nc.vector.tensor_add(
                            out=ot[:], in0=psums[m][:], in1=partials[(n, m)][:]
                        )
                        nc.sync.dma_start(
                            y[m * P : (m + 1) * P, n * NFREE : (n + 1) * NFREE],
                            ot[:],
                        )

    nc.compile()
    _nc_cache[key] = nc
    return nc


def _densify_wT(weight_blocks, block_rows, block_cols):
    """Scatter-add the 32x32 blocks into dense W^T [in_features, out_features]."""
    nc_blk = IN_F // BLOCK
    nr_blk = OUT_F // BLOCK
    wcr = np.zeros((nc_blk, nr_blk, BLOCK, BLOCK), np.float32)
    # block b occupies W[32r:32r+32, 32c:32c+32]; W^T gets the transposed block
    np.add.at(
        wcr,
        (block_cols.astype(np.int64), block_rows.astype(np.int64)),
        np.swapaxes(weight_blocks.astype(np.float32, copy=False), 1, 2),
    )
    return np.ascontiguousarray(wcr.transpose(0, 2, 1, 3).reshape(IN_F, OUT_F))


def kernel(x, weight_blocks, block_rows, block_cols):
    global _last_result
    from concourse.bass_utils import run_bass_kernel_spmd

    x = np.asarray(x, dtype=np.float32)
    wT = _densify_wT(
        np.asarray(weight_blocks), np.asarray(block_rows), np.asarray(block_cols)
    )
    xT = np.ascontiguousarray(x.T)

    in_maps = []
    for c in range(N_CORES):
        tg, og = divmod(c, OG)
        in_maps.append(
            {
                "xT": np.ascontiguousarray(xT[:, tg * T_SH : (tg + 1) * T_SH]),
                "wT": np.ascontiguousarray(wT[:, og * O_SH : (og + 1) * O_SH]),
            }
        )

    nc = _build_nc()
    res = run_bass_kernel_spmd(
        nc, in_maps, core_ids=list(range(N_CORES)), trace=TRACE
    )
    _last_result = res

    y = np.empty((TOKENS, OUT_F), np.float32)
    for c in range(N_CORES):
        tg, og = divmod(c, OG)
        y[tg * T_SH : (tg + 1) * T_SH, og * O_SH : (og + 1) * O_SH] = res.results[c][
            "y"
        ]
    return y
